# revision 1
# baseline (speedup 1.0000x reference)
"""MoE layer (8 experts, top-2) on 8 TRN2 NeuronCores, expert-parallel.

Strategy (sparse dispatch, per the sharding hint):
  - Core m owns expert m (w1[m], w2[m], b1[m], b2[m]).
  - Host computes top-2 expert ids per token (fp32 router, dispatch only)
    and "all-to-all"s: each core receives only the tokens routed to its
    expert, gathered as X_c^T [H, C] (C = max expert load, rounded to 128).
  - On device, each core re-runs the router (fp32 matmul on PE) over its
    gathered tokens and derives ITS OWN expert's combine weight per token
    purely elementwise:
        w_e(t) = exp(l_e - m1) / (1 + exp(m2 - m1))  if l_e >= m2 else 0
    (equals softmax-top2-renormalize of the reference).
  - FFN in bf16 (f32 PSUM accumulate): h1 = gelu(x @ w1 + b1) in [F, C]
    layout; y = (h1^T @ w2 + b2) * w with tokens on partitions -> yc [C, H].
  - Host scatter-adds each core's weighted outputs back to token order.
"""

from contextlib import ExitStack

import ml_dtypes
import numpy as np

P = 128
B, S, H, F, E = 2, 2048, 1024, 4096, 8
T = B * S            # 4096 tokens
KH = H // P          # 8   k-subtiles over H
KF = F // P          # 32  k-subtiles over F

_CACHE = {}


def _chunks(C):
    out = []
    t0 = 0
    while t0 < C:
        size = min(512, C - t0)
        out.append((t0, size))
        t0 += size
    return out


def _build_nc(C, reps=1):
    import concourse.mybir as mybir
    import concourse.tile as tile
    from concourse import bacc

    dt = mybir.dt
    AF = mybir.ActivationFunctionType
    ALU = mybir.AluOpType
    AX = mybir.AxisListType

    TTc = C // P  # token tiles

    nc = bacc.Bacc(
        "TRN2", target_bir_lowering=False, debug=False, num_devices=E)

    xct32 = nc.declare_dram_parameter("xct32", [H, C], dt.float32, isOutput=False)
    xctb = nc.declare_dram_parameter("xctb", [H, C], dt.bfloat16, isOutput=False)
    rw = nc.declare_dram_parameter("rw", [H, E], dt.float32, isOutput=False)
    rbb = nc.declare_dram_parameter("rbb", [P, E], dt.float32, isOutput=False)
    selb = nc.declare_dram_parameter("selb", [P, E], dt.float32, isOutput=False)
    w1d = nc.declare_dram_parameter("w1d", [H, F], dt.bfloat16, isOutput=False)
    w2d = nc.declare_dram_parameter("w2d", [F, H], dt.bfloat16, isOutput=False)
    b1d = nc.declare_dram_parameter("b1d", [P, KF], dt.float32, isOutput=False)
    b2b = nc.declare_dram_parameter("b2b", [P, H], dt.float32, isOutput=False)
    yc = nc.declare_dram_parameter("yc", [C, H], dt.float32, isOutput=True)

    xct32_r = xct32.rearrange("(k p) t -> p k t", p=P)
    xctb_r = xctb.rearrange("(k p) t -> p k t", p=P)
    rw_r = rw.rearrange("(k p) e -> p k e", p=P)
    w1_r = w1d.rearrange("(k p) f -> p k f", p=P)
    w2_r = w2d.rearrange("(k p) h -> p k h", p=P)

    with ExitStack() as ctx:
        tc = ctx.enter_context(tile.TileContext(nc))
        const = ctx.enter_context(tc.tile_pool(name="const", bufs=1))
        xrpool = ctx.enter_context(tc.tile_pool(name="xr", bufs=2))
        rpool = ctx.enter_context(tc.tile_pool(name="rtmp", bufs=3))
        rpsum = ctx.enter_context(tc.tile_pool(name="rpsum", bufs=1, space="PSUM"))
        xpool = ctx.enter_context(tc.tile_pool(name="xc", bufs=2))
        h1pool = ctx.enter_context(tc.tile_pool(name="h1", bufs=1))
        p1pool = ctx.enter_context(tc.tile_pool(name="p1", bufs=5, space="PSUM"))
        p2pool = ctx.enter_context(tc.tile_pool(name="p2", bufs=2, space="PSUM"))
        opool = ctx.enter_context(tc.tile_pool(name="ob", bufs=8))

        # Small constants first so nothing queues behind the weight stacks.
        # (b2b is 0.5MB and not needed until the first output stage ~70us in,
        # so it loads after the weight stream instead.)
        rbb_s = const.tile([P, E], dt.float32)
        nc.sync.dma_start(rbb_s[:], rbb[:])
        selb_s = const.tile([P, E], dt.float32)
        nc.sync.dma_start(selb_s[:], selb[:])
        b1_s = const.tile([P, KF], dt.float32)
        nc.sync.dma_start(b1_s[:], b1d[:])
        rw_s = const.tile([P, KH, E], dt.float32)
        b2b_s = const.tile([P, H], dt.float32)
        wmat = const.tile([P, TTc], dt.float32)

        chunks = _chunks(C)

        def load_xc(t0, csz):
            xc = xpool.tile([P, KH, 512], dt.bfloat16, name="xc")[:, :, :csz]
            for k in range(KH):
                nc.sync.dma_start(xc[:, k], xctb_r[:, k, t0:t0 + csz])
            return xc

        # Startup: interleave chunk-0 activations with w1's first f-chunk
        # per k so the first matmul group is runnable after ~2MB of DMA.
        # Then w1 f-chunk-major with w2 k-slices interleaved at a ratio
        # that keeps DMA just ahead of PE's w1 consumption, so w2 is
        # resident before chunk-0 matmul2 starts (~70us in).
        w1_s = const.tile([P, KH, F], dt.bfloat16)
        w2_s = const.tile([P, KF, H], dt.bfloat16)
        xc0 = xpool.tile([P, KH, 512], dt.bfloat16, name="xc")[:, :, :chunks[0][1]]
        for k in range(KH):
            nc.sync.dma_start(xc0[:, k], xctb_r[:, k, 0:chunks[0][1]])
            nc.sync.dma_start(w1_s[:, k, 0:512], w1_r[:, k, 0:512])
        w2_next = 0
        for fc in range(1, F // 512):
            for k in range(KH):
                nc.sync.dma_start(
                    w1_s[:, k, fc * 512:(fc + 1) * 512],
                    w1_r[:, k, fc * 512:(fc + 1) * 512])
            share = 0 if fc < 2 else (5 if fc < 7 else KF - w2_next)
            for k in range(w2_next, w2_next + share):
                nc.sync.dma_start(w2_s[:, k], w2_r[:, k])
            w2_next += share
            if fc == 4:
                nc.sync.dma_start(rw_s[:], rw_r)
        nc.sync.dma_start(b2b_s[:], b2b[:])

        def emit_mm1(xc, csz):
            h1 = h1pool.tile([P, KF, 512], dt.bfloat16, name="h1")[:, :, :csz]
            for f in range(KF):
                ps1 = p1pool.tile([P, 512], dt.float32, name="ps1")[:, :csz]
                for k in range(KH):
                    nc.tensor.matmul(
                        ps1[:], w1_s[:, k, f * P:(f + 1) * P], xc[:, k],
                        start=(k == 0), stop=(k == KH - 1),
                    )
                nc.scalar.activation(h1[:, f], ps1[:], AF.Gelu, bias=b1_s[:, f:f + 1])
            return h1

        def emit_mm2(h1, t0, csz, tail_split=False):
            for ct in range(csz // P):
                gt = t0 // P + ct
                for hh in range(H // 512):
                    last = tail_split and ct == csz // P - 1 and hh == H // 512 - 1
                    # The very last group splits in two halves so its output
                    # pipeline (DVE + DMA) overlaps the second half's matmuls
                    # instead of running serially after PE finishes.
                    for (o0, wid) in ([(0, 256), (256, 128), (384, 64), (448, 64)] if last else [(0, 512)]):
                        ps2 = p2pool.tile([P, 512], dt.float32, name="ps2")[:, :wid]
                        for k in range(KF):
                            nc.tensor.matmul(
                                ps2[:], h1[:, k, ct * P:(ct + 1) * P],
                                w2_s[:, k, hh * 512 + o0:hh * 512 + o0 + wid],
                                start=(k == 0), stop=(k == KF - 1),
                            )
                        ob = opool.tile([P, 512], dt.float32, name="ob")[:, :wid]
                        nc.vector.tensor_tensor(
                            ob[:], ps2[:],
                            b2b_s[:, hh * 512 + o0:hh * 512 + o0 + wid], ALU.add)
                        nc.vector.tensor_scalar_mul(ob[:], ob[:], wmat[:, gt:gt + 1])
                        nc.sync.dma_start(
                            yc[gt * P:(gt + 1) * P,
                               hh * 512 + o0:hh * 512 + o0 + wid], ob[:])

        for _rep in range(reps):
            # Chunk-0 first FFN matmul overlaps the router's DMAs.
            h1_0 = emit_mm1(xc0, chunks[0][1])

            # ---- Router: combine weight of MY expert for my gathered tokens ----
            for tt in range(TTc):
                xt_t = xrpool.tile([P, KH, P], dt.float32)
                nc.sync.dma_start(xt_t[:], xct32_r[:, :, tt * P:(tt + 1) * P])
                lg = rpsum.tile([P, E], dt.float32)
                for k in range(KH):
                    nc.tensor.matmul(
                        lg[:], xt_t[:, k], rw_s[:, k],
                        start=(k == 0), stop=(k == KH - 1),
                    )
                l = rpool.tile([P, E], dt.float32)
                nc.vector.tensor_tensor(l[:], lg[:], rbb_s[:], ALU.add)
                m1 = rpool.tile([P, 1], dt.float32)
                nc.vector.reduce_max(m1[:], l[:], axis=AX.X)
                nm1 = rpool.tile([P, 1], dt.float32)
                nc.vector.tensor_scalar_mul(nm1[:], m1[:], -1.0)
                ismax = rpool.tile([P, E], dt.float32)
                nc.vector.tensor_tensor(
                    ismax[:], l[:], m1[:].to_broadcast((P, E)), ALU.is_equal)
                pen = rpool.tile([P, E], dt.float32)
                nc.vector.tensor_scalar_mul(pen[:], ismax[:], 1e30)
                lmask = rpool.tile([P, E], dt.float32)
                nc.vector.tensor_tensor(lmask[:], l[:], pen[:], ALU.subtract)
                m2 = rpool.tile([P, 1], dt.float32)
                nc.vector.reduce_max(m2[:], lmask[:], axis=AX.X)
                lsel = rpool.tile([P, E], dt.float32)
                nc.vector.tensor_tensor(lsel[:], l[:], selb_s[:], ALU.mult)
                lmine = rpool.tile([P, 1], dt.float32)
                nc.vector.reduce_sum(lmine[:], lsel[:], axis=AX.X)
                ge = rpool.tile([P, 1], dt.float32)
                nc.vector.tensor_tensor(ge[:], lmine[:], m2[:], ALU.is_ge)
                e1 = rpool.tile([P, 1], dt.float32)
                nc.scalar.activation(e1[:], lmine[:], AF.Exp, bias=nm1[:])
                e2 = rpool.tile([P, 1], dt.float32)
                nc.scalar.activation(e2[:], m2[:], AF.Exp, bias=nm1[:])
                den = rpool.tile([P, 1], dt.float32)
                nc.vector.tensor_scalar_add(den[:], e2[:], 1.0)
                rec = rpool.tile([P, 1], dt.float32)
                nc.vector.reciprocal(rec[:], den[:])
                wnum = rpool.tile([P, 1], dt.float32)
                nc.vector.tensor_tensor(wnum[:], e1[:], ge[:], ALU.mult)
                nc.vector.tensor_tensor(wmat[:, tt:tt + 1], wnum[:], rec[:], ALU.mult)

            # ---- Expert FFN over gathered tokens, weighted output ----
            emit_mm2(h1_0, chunks[0][0], chunks[0][1],
                     tail_split=(len(chunks) == 1))
            for ci, (t0, csz) in enumerate(chunks[1:], start=1):
                xc = load_xc(t0, csz)
                h1 = emit_mm1(xc, csz)
                emit_mm2(h1, t0, csz, tail_split=(ci == len(chunks) - 1))
    return nc


def _get_nc(C, reps=1):
    key = (C, reps)
    if key not in _CACHE:
        nc = _build_nc(C, reps)
        nc.finalize()
        _CACHE[key] = nc
    return _CACHE[key]


def dispatch(hidden_states, router_w, router_b):
    """Host-side top-2 dispatch: per-expert token index lists + capacity."""
    x = np.asarray(hidden_states, dtype=np.float32).reshape(T, H)
    logits = x @ np.asarray(router_w, dtype=np.float32)
    logits = logits + np.asarray(router_b, dtype=np.float32)
    top2 = np.argpartition(logits, E - 2, axis=1)[:, E - 2:]  # [T, 2] unordered
    idx_lists = []
    for m in range(E):
        idx_lists.append(np.where((top2 == m).any(axis=1))[0])
    cmax = max(len(ix) for ix in idx_lists)
    C = max(P, ((cmax + P - 1) // P) * P)
    return x, idx_lists, C


def make_in_maps(hidden_states, router_w, router_b, w1, b1, w2, b2):
    bf16 = ml_dtypes.bfloat16
    x, idx_lists, C = dispatch(hidden_states, router_w, router_b)
    xt = np.ascontiguousarray(x.T)            # [H, T] f32
    xtb = xt.astype(bf16)
    rw = np.ascontiguousarray(np.asarray(router_w, dtype=np.float32))
    rbb = np.ascontiguousarray(
        np.broadcast_to(np.asarray(router_b, dtype=np.float32), (P, E)))
    w1 = np.asarray(w1, dtype=np.float32)
    w2 = np.asarray(w2, dtype=np.float32)
    b1 = np.asarray(b1, dtype=np.float32)
    b2 = np.asarray(b2, dtype=np.float32)
    in_maps = []
    for m in range(E):
        ix = idx_lists[m]
        pad = np.zeros(C, dtype=np.int64)
        pad[:len(ix)] = ix
        sel = np.zeros((P, E), dtype=np.float32)
        sel[:, m] = 1.0
        in_maps.append({
            "xct32": np.ascontiguousarray(xt[:, pad]),
            "xctb": np.ascontiguousarray(xtb[:, pad]),
            "rw": rw,
            "rbb": rbb,
            "selb": sel,
            "w1d": np.ascontiguousarray(w1[m].astype(bf16)),
            "w2d": np.ascontiguousarray(w2[m].astype(bf16)),
            "b1d": np.ascontiguousarray(b1[m].reshape(KF, P).T),
            "b2b": np.ascontiguousarray(np.broadcast_to(b2[m], (P, H))),
        })
    return in_maps, idx_lists, C


def run_device(in_maps, C):
    from concourse.bass_utils import run_bass_kernel_spmd

    nc = _get_nc(C)
    res = run_bass_kernel_spmd(nc, in_maps, core_ids=list(range(E)))
    return res.results


def kernel(hidden_states, router_w, router_b, w1, b1, w2, b2):
    in_maps, idx_lists, C = make_in_maps(
        hidden_states, router_w, router_b, w1, b1, w2, b2)
    # One retry guards against a rare transient execution glitch observed on
    # the very first load of a freshly compiled NEFF (garbage ~1e35 values);
    # a healthy output has absmax of a few units.
    last_err = None
    for attempt in range(3):
        try:
            results = run_device(in_maps, C)
        except Exception as e:  # transient NRT/axon failures observed
            last_err = e
            import time as _time
            _time.sleep(10)
            continue
        acc = np.zeros((T, H), dtype=np.float32)
        for m in range(E):
            ix = idx_lists[m]
            acc[ix] += np.asarray(results[m]["yc"], dtype=np.float32)[:len(ix)]
        if np.isfinite(acc).all() and np.abs(acc).max() < 1e4:
            return acc.reshape(B, S, H)
    if last_err is not None:
        raise last_err
    return acc.reshape(B, S, H)



# revision 2
# speedup vs baseline: 1.2616x; 1.2616x over previous
"""MoE layer (8 experts, top-2) on 8 TRN2 NeuronCores, expert-parallel.

Strategy (sparse dispatch; fp8 DoubleRow matmuls with hi/lo residual split):
  - Core m owns expert m (w1[m], w2[m]).  Host computes the router exactly
    (softmax top-2 renormalize) and gathers each expert's tokens as
    X^T [H, C] (C = max expert load, rounded to 16*nchunks).  Combine
    weights are applied on the host during scatter-add, so the device runs
    a pure dense FFN over the gathered tokens.
  - Precision: every operand is split into hi + lo fp8e4m3 parts
    (a = ah + al with ah = fp8(a*s), al = fp8(a*s - ah)).  Each logical
    matmul a @ b is computed as ah@bh + al@bh + ah@bl (dropping al@bl),
    which lands near-bf16 accuracy (~2e-3 end-to-end rel err) while every
    term is an fp8 DoubleRow matmul (K=256 per instruction at 0.5
    cycles/row -> 4x the bf16 FLOP rate; 3 terms -> net 0.75x bf16 time).
  - Layouts: x packed [H, 2, C] (hi, lo); weights packed (lo, hi) so a
    single DoubleRow instruction computes BOTH cross terms of one k-tile:
    pair (xh, xl) against (wl, wh) gives xh@wl + xl@wh.
  - mm1: h[f, t] = gelu(x @ w1 + b1) with f on partitions, tokens on the
    free dim; gelu output split hi/lo on device (ACT gelu + DVE copy/sub).
  - mm2: y[h, t] = h @ w2 with h on partitions, tokens free, so capacity
    needs no 128-token padding (cost is linear in C).
  - Host scatter-adds y^T * combine_weight (+ b2) back to token order.
"""

from contextlib import ExitStack

import ml_dtypes
import numpy as np

P = 128
B, S, H, F, E = 2, 2048, 1024, 4096, 8
T = B * S            # 4096 tokens
KH = H // P          # 8   k-subtiles over H
KF = F // P          # 32  k-subtiles over F

SX = 32.0            # x fp8 scale
SW1 = 1024.0         # w1 fp8 scale
SW2 = 1024.0         # w2 fp8 scale (h is quantized at scale 1)

E4NP = ml_dtypes.float8_e4m3   # TRN fp8e4 (max normal 240)

_CACHE = {}


def _split_hl(a, scale):
    """Split a into (hi, lo) e4m3 parts of a*scale: a*scale ~ hi + lo."""
    s = (np.asarray(a, dtype=np.float32) * np.float32(scale))
    hi = s.astype(E4NP)
    lo = (s - hi.astype(np.float32)).astype(E4NP)
    return hi, lo


def _chunking(c0):
    nchunks = max(1, -(-c0 // 512))
    chunk = -(-(-(-c0 // nchunks)) // 16) * 16
    return chunk * nchunks, chunk, nchunks


def _build_nc(C, chunk, nchunks):
    import concourse.mybir as mybir
    import concourse.tile as tile
    from concourse import bacc

    dt = mybir.dt
    AF = mybir.ActivationFunctionType
    ALU = mybir.AluOpType
    DR = mybir.MatmulPerfMode.DoubleRow

    CR = C - chunk  # columns beyond chunk 0

    nc = bacc.Bacc(
        "TRN2", target_bir_lowering=False, debug=False, num_devices=E)

    # x: [H, 2, cols] with (hi, lo) on dim1; weights packed (lo, hi).
    xhl0 = nc.declare_dram_parameter("xhl0", [H, 2, chunk], dt.float8e4, isOutput=False)
    if CR:
        xhlr = nc.declare_dram_parameter("xhlr", [H, 2, CR], dt.float8e4, isOutput=False)
    w1lh = nc.declare_dram_parameter("w1lh", [H, 2, F], dt.float8e4, isOutput=False)
    w2lh = nc.declare_dram_parameter("w2lh", [F, 2, H], dt.float8e4, isOutput=False)
    b1d = nc.declare_dram_parameter("b1d", [P, KF], dt.float32, isOutput=False)
    yc = nc.declare_dram_parameter("yc", [H, C], dt.float32, isOutput=True)

    x0_r = xhl0.rearrange("(k p) two t -> p k two t", p=P)
    if CR:
        xr_r = xhlr.rearrange("(k p) two t -> p k two t", p=P)
    w1_r = w1lh.rearrange("(k p) two f -> p k two f", p=P)
    w2_r = w2lh.rearrange("(k p) two h -> p k two h", p=P)

    with ExitStack() as ctx:
        tc = ctx.enter_context(tile.TileContext(nc))
        const = ctx.enter_context(tc.tile_pool(name="const", bufs=1))
        h1pool = ctx.enter_context(tc.tile_pool(name="h1", bufs=min(2, nchunks)))
        hfpool = ctx.enter_context(tc.tile_pool(name="hf", bufs=2))
        p1pool = ctx.enter_context(tc.tile_pool(name="p1", bufs=3, space="PSUM"))
        p2pool = ctx.enter_context(tc.tile_pool(name="p2", bufs=3, space="PSUM"))
        opool = ctx.enter_context(tc.tile_pool(name="ob", bufs=3))

        b1_s = const.tile([P, KF], dt.float32)
        nc.sync.dma_start(b1_s[:], b1d[:])

        x0_s = const.tile([P, KH, 2, chunk], dt.float8e4)
        for k in range(KH):
            nc.sync.dma_start(x0_s[:, k], x0_r[:, k])

        w1_s = const.tile([P, KH, 2, F], dt.float8e4)
        w2_s = const.tile([P, KF, 2, H], dt.float8e4)
        if CR:
            xr_s = const.tile([P, KH, 2, CR], dt.float8e4)

        # w1 stream f-chunk-major so PE can start after ~1MB; x for chunk 1
        # rides in the middle; w2 streams afterwards (first needed by mm2 of
        # chunk 0, which starts two PE phases in).
        for fc in range(F // 512):
            for k in range(KH):
                nc.sync.dma_start(
                    w1_s[:, k, :, fc * 512:(fc + 1) * 512],
                    w1_r[:, k, :, fc * 512:(fc + 1) * 512])
            if fc == 2 and CR:
                for k in range(KH):
                    nc.sync.dma_start(xr_s[:, k], xr_r[:, k])
        for k in range(KF):
            nc.sync.dma_start(w2_s[:, k], w2_r[:, k])

        def xs_main(ci, kp):      # (xh[2kp], xh[2kp+1]) moving pair
            if ci == 0:
                return x0_s[:, 2 * kp:2 * kp + 2, 0, :]
            t0 = ci * chunk - chunk
            return xr_s[:, 2 * kp:2 * kp + 2, 0, t0:t0 + chunk]

        def xs_corr(ci, k):       # (xh[k], xl[k]) moving pair
            if ci == 0:
                return x0_s[:, k, :, :]
            t0 = ci * chunk - chunk
            return xr_s[:, k, :, t0:t0 + chunk]

        h1s = [None] * nchunks

        def emit_mm1(ci):
            h1 = h1pool.tile([P, KF, 2, chunk], dt.float8e4, name="h1")
            h1s[ci] = h1
            for f in range(KF):
                ps1 = p1pool.tile([P, 512], dt.float32, name="ps1")[:, :chunk]
                fs = slice(f * P, (f + 1) * P)
                for kp in range(KH // 2):   # main: xh . w1h
                    nc.tensor.matmul(
                        ps1[:], w1_s[:, 2 * kp:2 * kp + 2, 1, fs], xs_main(ci, kp),
                        start=(kp == 0), stop=False, perf_mode=DR)
                for k in range(KH):         # cross: xh . w1l + xl . w1h
                    nc.tensor.matmul(
                        ps1[:], w1_s[:, k, :, fs], xs_corr(ci, k),
                        start=False, stop=(k == KH - 1), perf_mode=DR)
                hf = hfpool.tile([P, 512], dt.float32, name="hf")[:, :chunk]
                nc.scalar.activation(
                    hf[:], ps1[:], AF.Gelu,
                    bias=b1_s[:, f:f + 1], scale=1.0 / (SX * SW1))
                nc.vector.tensor_copy(h1[:, f, 0, :], hf[:])
                nc.vector.tensor_tensor(
                    h1[:, f, 1, :], hf[:], h1[:, f, 0, :], ALU.subtract)

        def emit_mm2(ci):
            h1 = h1s[ci]
            t0 = ci * chunk
            for hh in range(H // P):
                ps2 = p2pool.tile([P, 512], dt.float32, name="ps2")[:, :chunk]
                hs = slice(hh * P, (hh + 1) * P)
                for kp in range(KF // 2):   # main: h_hi . w2h
                    nc.tensor.matmul(
                        ps2[:], w2_s[:, 2 * kp:2 * kp + 2, 1, hs],
                        h1[:, 2 * kp:2 * kp + 2, 0, :],
                        start=(kp == 0), stop=False, perf_mode=DR)
                for k in range(KF):         # cross terms
                    nc.tensor.matmul(
                        ps2[:], w2_s[:, k, :, hs], h1[:, k, :, :],
                        start=False, stop=(k == KF - 1), perf_mode=DR)
                ob = opool.tile([P, 512], dt.float32, name="ob")[:, :chunk]
                nc.vector.tensor_copy(ob[:], ps2[:])
                nc.sync.dma_start(yc[hs, t0:t0 + chunk], ob[:])

        # PE phase order: run two mm1 phases ahead so w2's DMA stream (and
        # each chunk's gelu/split chain) completes before its mm2 needs it.
        emit_mm1(0)
        if nchunks > 1:
            emit_mm1(1)
        for ci in range(nchunks):
            emit_mm2(ci)
            if ci + 2 < nchunks:
                emit_mm1(ci + 2)
    return nc


def _get_nc(C, reps=1):
    Cr, chunk, nchunks = _chunking(C)
    assert Cr == C, (C, Cr)
    key = (C, reps)
    if key not in _CACHE:
        nc = _build_nc(C, chunk, nchunks)
        nc.finalize()
        _CACHE[key] = nc
    return _CACHE[key]


def dispatch(hidden_states, router_w, router_b):
    """Host router: exact top-2 ids + renormalized softmax weights."""
    x = np.asarray(hidden_states, dtype=np.float32).reshape(T, H)
    logits = x @ np.asarray(router_w, dtype=np.float32)
    logits = logits + np.asarray(router_b, dtype=np.float32)
    top2 = np.argsort(-logits, axis=1, kind="stable")[:, :2]      # [T, 2]
    l2 = np.take_along_axis(logits, top2, 1)
    e = np.exp(l2 - l2.max(1, keepdims=True))
    wts = (e / e.sum(1, keepdims=True)).astype(np.float32)        # [T, 2]
    combines = []
    for m in range(E):
        sel = top2 == m
        ix = np.where(sel.any(axis=1))[0]
        w_tok = np.where(sel[ix, 0], wts[ix, 0], wts[ix, 1])
        combines.append((ix, w_tok.astype(np.float32)))
    c0 = max(len(ix) for ix, _ in combines)
    C, chunk, nchunks = _chunking(c0)
    return x, combines, C


def make_in_maps(hidden_states, router_w, router_b, w1, b1, w2, b2):
    x, combines, C = dispatch(hidden_states, router_w, router_b)
    chunk = _chunking(C)[1]
    xt = np.ascontiguousarray(x.T)                  # [H, T] f32
    xh8, xl8 = _split_hl(xt, SX)                    # [H, T] e4m3 each
    w1 = np.asarray(w1, dtype=np.float32)
    w2 = np.asarray(w2, dtype=np.float32)
    b1 = np.asarray(b1, dtype=np.float32)
    in_maps = []
    for m in range(E):
        ix, _ = combines[m]
        pad = np.zeros(C, dtype=np.int64)
        pad[:len(ix)] = ix
        xhl = np.empty((H, 2, C), dtype=E4NP)
        xhl[:, 0, :] = xh8[:, pad]
        xhl[:, 1, :] = xl8[:, pad]
        w1h, w1l = _split_hl(w1[m], SW1)            # [H, F]
        w2h, w2l = _split_hl(w2[m], SW2)            # [F, H]
        im = {
            "xhl0": np.ascontiguousarray(xhl[:, :, :chunk]),
            "w1lh": np.ascontiguousarray(np.stack([w1l, w1h], axis=1)),
            "w2lh": np.ascontiguousarray(np.stack([w2l, w2h], axis=1)),
            "b1d": np.ascontiguousarray(b1[m].reshape(KF, P).T),
        }
        if C > chunk:
            im["xhlr"] = np.ascontiguousarray(xhl[:, :, chunk:])
        in_maps.append(im)
    return in_maps, combines, C


def run_device(in_maps, C):
    from concourse.bass_utils import run_bass_kernel_spmd

    nc = _get_nc(C)
    res = run_bass_kernel_spmd(nc, in_maps, core_ids=list(range(E)))
    return res.results


def kernel(hidden_states, router_w, router_b, w1, b1, w2, b2):
    in_maps, combines, C = make_in_maps(
        hidden_states, router_w, router_b, w1, b1, w2, b2)
    b2 = np.asarray(b2, dtype=np.float32)
    # One retry guards against a rare transient execution glitch observed on
    # the very first load of a freshly compiled NEFF (garbage ~1e35 values);
    # a healthy output has absmax of a few units.
    last_err = None
    for attempt in range(3):
        try:
            results = run_device(in_maps, C)
        except Exception as e:  # transient NRT/axon failures observed
            last_err = e
            import time as _time
            _time.sleep(10)
            continue
        acc = np.zeros((T, H), dtype=np.float32)
        for m in range(E):
            ix, w_tok = combines[m]
            ym = np.asarray(results[m]["yc"], dtype=np.float32)  # [H, C]
            acc[ix] += w_tok[:, None] * (
                ym[:, :len(ix)].T * np.float32(1.0 / SW2) + b2[m])
        if np.isfinite(acc).all() and np.abs(acc).max() < 1e4:
            return acc.reshape(B, S, H)
    if last_err is not None:
        raise last_err
    return acc.reshape(B, S, H)


# revision 8
# speedup vs baseline: 1.3624x; 1.0799x over previous
"""MoE layer (8 experts, top-2) on 8 TRN2 NeuronCores, expert-parallel.

Strategy (sparse dispatch; fp8 DoubleRow matmuls with hi/lo residual split):
  - Core m owns expert m (w1[m], w2[m]).  Host computes the router exactly
    (softmax top-2 renormalize) and gathers each expert's tokens as
    X^T [H, C] (C = max expert load, rounded to 16*nchunks).  Combine
    weights are applied on the host during scatter-add, so the device runs
    a pure dense FFN over the gathered tokens.
  - Precision: every operand is split into hi + lo fp8e4m3 parts
    (a = ah + al with ah = fp8(a*s), al = fp8(a*s - ah)).  Each logical
    matmul a @ b is computed as ah@bh + al@bh + ah@bl (dropping al@bl),
    which lands near-bf16 accuracy (~2e-3 end-to-end rel err) while every
    term is an fp8 DoubleRow matmul (K=256 per instruction at 0.5
    cycles/row -> 4x the bf16 FLOP rate; 3 terms -> net 0.75x bf16 time).
  - Layouts: x packed [H, 2, C] (hi, lo); weights packed (lo, hi) so a
    single DoubleRow instruction computes BOTH cross terms of one k-tile:
    pair (xh, xl) against (wl, wh) gives xh@wl + xl@wh.
  - mm1: h[f, t] = gelu(x @ w1 + b1) with f on partitions, tokens on the
    free dim; gelu output split hi/lo on device (ACT gelu + DVE copy/sub).
  - mm2: y[h, t] = h @ w2 with h on partitions, tokens free, so capacity
    needs no 128-token padding (cost is linear in C).
  - Host scatter-adds y^T * combine_weight (+ b2) back to token order.
"""

from contextlib import ExitStack

import ml_dtypes
import numpy as np

P = 128
B, S, H, F, E = 2, 2048, 1024, 4096, 8
T = B * S            # 4096 tokens
KH = H // P          # 8   k-subtiles over H
KF = F // P          # 32  k-subtiles over F

SX = 32.0            # x fp8 scale
SW1 = 1024.0         # w1 fp8 scale
SW2 = 1024.0         # w2 fp8 scale (h is quantized at scale 1)

E4NP = ml_dtypes.float8_e4m3   # TRN fp8e4 (max normal 240)

_CACHE = {}


def _split_hl(a, scale):
    """Split a into (hi, lo) e4m3 parts of a*scale: a*scale ~ hi + lo."""
    s = (np.asarray(a, dtype=np.float32) * np.float32(scale))
    hi = s.astype(E4NP)
    lo = (s - hi.astype(np.float32)).astype(E4NP)
    return hi, lo


def _chunking(c0):
    nchunks = max(1, -(-c0 // 512))
    chunk = -(-(-(-c0 // nchunks)) // 16) * 16
    return chunk * nchunks, chunk, nchunks


def _build_nc(C, chunk, nchunks):
    import concourse.mybir as mybir
    import concourse.tile as tile
    from concourse import bacc

    dt = mybir.dt
    AF = mybir.ActivationFunctionType
    ALU = mybir.AluOpType
    DR = mybir.MatmulPerfMode.DoubleRow

    CR = C - chunk  # columns beyond chunk 0

    nc = bacc.Bacc(
        "TRN2", target_bir_lowering=False, debug=False, num_devices=E)

    # x: [H, 2, cols] with (hi, lo) on dim1; weights packed (lo, hi).
    FC = F // 512   # w1 f-chunks (DMA granularity)
    xhl0 = nc.declare_dram_parameter("xhl0", [H, 2, chunk], dt.float8e4, isOutput=False)
    if CR:
        xhlr = nc.declare_dram_parameter("xhlr", [H, 2, CR], dt.float8e4, isOutput=False)
    w1lh = nc.declare_dram_parameter("w1lh", [FC, H, 2, 512], dt.float8e4, isOutput=False)
    w2lh = nc.declare_dram_parameter("w2lh", [F, 2, H], dt.float8e4, isOutput=False)
    b1d = nc.declare_dram_parameter("b1d", [P, KF], dt.float32, isOutput=False)
    yc = nc.declare_dram_parameter("yc", [H, C], dt.float32, isOutput=True)

    x0_r = xhl0.rearrange("(k p) two t -> p k two t", p=P)
    if CR:
        xr_r = xhlr.rearrange("(k p) two t -> p k two t", p=P)
    w1_r = w1lh.rearrange("c (k p) two f -> p c k two f", p=P)
    w2_r = w2lh.rearrange("(k p) two h -> p k two h", p=P)

    with ExitStack() as ctx:
        tc = ctx.enter_context(tile.TileContext(nc))
        const = ctx.enter_context(tc.tile_pool(name="const", bufs=1))
        h1pool = ctx.enter_context(tc.tile_pool(name="h1", bufs=min(2, nchunks)))
        hfpool = ctx.enter_context(tc.tile_pool(name="hf", bufs=4))
        p1pool = ctx.enter_context(tc.tile_pool(name="p1", bufs=3, space="PSUM"))
        p2pool = ctx.enter_context(tc.tile_pool(name="p2", bufs=3, space="PSUM"))
        opool = ctx.enter_context(tc.tile_pool(name="ob", bufs=3))

        b1_s = const.tile([P, KF], dt.float32)
        nc.sync.dma_start(b1_s[:], b1d[:])

        x0_s = const.tile([P, KH, 2, chunk], dt.float8e4)
        nc.sync.dma_start(x0_s[:], x0_r[:])

        w1_s = const.tile([P, FC, KH, 2, 512], dt.float8e4)
        w2_s = const.tile([P, KF, 2, H], dt.float8e4)
        if CR:
            xr_s = const.tile([P, KH, 2, CR], dt.float8e4)

        # Batched DMAs (one instruction per slab — the SP sequencer issues a
        # DMA only every ~565ns, so many small DMAs would starve the PE).
        # w1 streams f-chunk-major so PE can start after ~1MB; x for chunks
        # 1+ rides in the middle; w2 streams afterwards (first needed by mm2
        # of chunk 0, which starts two PE phases in).
        for fc in range(FC):
            nc.sync.dma_start(w1_s[:, fc], w1_r[:, fc])
            if fc == 2 and CR:
                nc.sync.dma_start(xr_s[:], xr_r[:])
        nc.sync.dma_start(w2_s[:], w2_r[:])

        def xs_main(ci, kp):      # (xh[2kp], xh[2kp+1]) moving pair
            if ci == 0:
                return x0_s[:, 2 * kp:2 * kp + 2, 0, :]
            t0 = ci * chunk - chunk
            return xr_s[:, 2 * kp:2 * kp + 2, 0, t0:t0 + chunk]

        def xs_corr(ci, k):       # (xh[k], xl[k]) moving pair
            if ci == 0:
                return x0_s[:, k, :, :]
            t0 = ci * chunk - chunk
            return xr_s[:, k, :, t0:t0 + chunk]

        h1s = [None] * nchunks

        def emit_mm1(ci):
            h1 = h1pool.tile([P, KF, 2, chunk], dt.float8e4, name="h1")
            h1s[ci] = h1
            for f in range(KF):
                ps1 = p1pool.tile([P, 512], dt.float32, name="ps1")[:, :chunk]
                fc, fi = divmod(f, 4)
                fs = slice(fi * P, (fi + 1) * P)
                for kp in range(KH // 2):   # main: xh . w1h
                    nc.tensor.matmul(
                        ps1[:], w1_s[:, fc, 2 * kp:2 * kp + 2, 1, fs],
                        xs_main(ci, kp),
                        start=(kp == 0), stop=False, perf_mode=DR)
                for k in range(KH):         # cross: xh . w1l + xl . w1h
                    nc.tensor.matmul(
                        ps1[:], w1_s[:, fc, k, :, fs], xs_corr(ci, k),
                        start=False, stop=(k == KH - 1), perf_mode=DR)
                hf = hfpool.tile([P, 512], dt.float32, name="hf")[:, :chunk]
                nc.scalar.activation(
                    hf[:], ps1[:], AF.Gelu,
                    bias=b1_s[:, f:f + 1], scale=1.0 / (SX * SW1))
                nc.vector.tensor_copy(h1[:, f, 0, :], hf[:])
                nc.vector.tensor_tensor(
                    h1[:, f, 1, :], hf[:], h1[:, f, 0, :], ALU.subtract)

        def emit_mm2(ci):
            h1 = h1s[ci]
            t0 = ci * chunk
            for hh in range(H // P):
                ps2 = p2pool.tile([P, 512], dt.float32, name="ps2")[:, :chunk]
                hs = slice(hh * P, (hh + 1) * P)
                for kp in range(KF // 2):   # main: h_hi . w2h
                    nc.tensor.matmul(
                        ps2[:], w2_s[:, 2 * kp:2 * kp + 2, 1, hs],
                        h1[:, 2 * kp:2 * kp + 2, 0, :],
                        start=(kp == 0), stop=False, perf_mode=DR)
                for k in range(KF):         # cross terms
                    nc.tensor.matmul(
                        ps2[:], w2_s[:, k, :, hs], h1[:, k, :, :],
                        start=False, stop=(k == KF - 1), perf_mode=DR)
                ob = opool.tile([P, 512], dt.float32, name="ob")[:, :chunk]
                nc.vector.tensor_copy(ob[:], ps2[:])
                nc.sync.dma_start(yc[hs, t0:t0 + chunk], ob[:])

        # PE phase order: run two mm1 phases ahead so w2's DMA stream (and
        # each chunk's gelu/split chain) completes before its mm2 needs it.
        emit_mm1(0)
        if nchunks > 1:
            emit_mm1(1)
        for ci in range(nchunks):
            emit_mm2(ci)
            if ci + 2 < nchunks:
                emit_mm1(ci + 2)
    return nc


def _get_nc(C, reps=1):
    Cr, chunk, nchunks = _chunking(C)
    assert Cr == C, (C, Cr)
    key = (C, reps)
    if key not in _CACHE:
        nc = _build_nc(C, chunk, nchunks)
        nc.finalize()
        _CACHE[key] = nc
    return _CACHE[key]


def dispatch(hidden_states, router_w, router_b):
    """Host router: exact top-2 ids + renormalized softmax weights."""
    x = np.asarray(hidden_states, dtype=np.float32).reshape(T, H)
    logits = x @ np.asarray(router_w, dtype=np.float32)
    logits = logits + np.asarray(router_b, dtype=np.float32)
    top2 = np.argsort(-logits, axis=1, kind="stable")[:, :2]      # [T, 2]
    l2 = np.take_along_axis(logits, top2, 1)
    e = np.exp(l2 - l2.max(1, keepdims=True))
    wts = (e / e.sum(1, keepdims=True)).astype(np.float32)        # [T, 2]
    combines = []
    for m in range(E):
        sel = top2 == m
        ix = np.where(sel.any(axis=1))[0]
        w_tok = np.where(sel[ix, 0], wts[ix, 0], wts[ix, 1])
        combines.append((ix, w_tok.astype(np.float32)))
    c0 = max(len(ix) for ix, _ in combines)
    C, chunk, nchunks = _chunking(c0)
    return x, combines, C


def make_in_maps(hidden_states, router_w, router_b, w1, b1, w2, b2):
    x, combines, C = dispatch(hidden_states, router_w, router_b)
    chunk = _chunking(C)[1]
    xt = np.ascontiguousarray(x.T)                  # [H, T] f32
    xh8, xl8 = _split_hl(xt, SX)                    # [H, T] e4m3 each
    w1 = np.asarray(w1, dtype=np.float32)
    w2 = np.asarray(w2, dtype=np.float32)
    b1 = np.asarray(b1, dtype=np.float32)
    in_maps = []
    for m in range(E):
        ix, _ = combines[m]
        pad = np.zeros(C, dtype=np.int64)
        pad[:len(ix)] = ix
        xhl = np.empty((H, 2, C), dtype=E4NP)
        xhl[:, 0, :] = xh8[:, pad]
        xhl[:, 1, :] = xl8[:, pad]
        w1h, w1l = _split_hl(w1[m], SW1)            # [H, F]
        w2h, w2l = _split_hl(w2[m], SW2)            # [F, H]
        w1lh = np.stack([w1l, w1h], axis=1)         # [H, 2, F]
        w1lh = w1lh.reshape(H, 2, F // 512, 512).transpose(2, 0, 1, 3)
        im = {
            "xhl0": np.ascontiguousarray(xhl[:, :, :chunk]),
            "w1lh": np.ascontiguousarray(w1lh),
            "w2lh": np.ascontiguousarray(np.stack([w2l, w2h], axis=1)),
            "b1d": np.ascontiguousarray(b1[m].reshape(KF, P).T),
        }
        if C > chunk:
            im["xhlr"] = np.ascontiguousarray(xhl[:, :, chunk:])
        in_maps.append(im)
    return in_maps, combines, C


def run_device(in_maps, C):
    from concourse.bass_utils import run_bass_kernel_spmd

    nc = _get_nc(C)
    res = run_bass_kernel_spmd(nc, in_maps, core_ids=list(range(E)))
    return res.results


def kernel(hidden_states, router_w, router_b, w1, b1, w2, b2):
    in_maps, combines, C = make_in_maps(
        hidden_states, router_w, router_b, w1, b1, w2, b2)
    b2 = np.asarray(b2, dtype=np.float32)
    # One retry guards against a rare transient execution glitch observed on
    # the very first load of a freshly compiled NEFF (garbage ~1e35 values);
    # a healthy output has absmax of a few units.
    last_err = None
    for attempt in range(3):
        try:
            results = run_device(in_maps, C)
        except Exception as e:  # transient NRT/axon failures observed
            last_err = e
            import time as _time
            _time.sleep(10)
            continue
        acc = np.zeros((T, H), dtype=np.float32)
        for m in range(E):
            ix, w_tok = combines[m]
            ym = np.asarray(results[m]["yc"], dtype=np.float32)  # [H, C]
            acc[ix] += w_tok[:, None] * (
                ym[:, :len(ix)].T * np.float32(1.0 / SW2) + b2[m])
        if np.isfinite(acc).all() and np.abs(acc).max() < 1e4:
            return acc.reshape(B, S, H)
    if last_err is not None:
        raise last_err
    return acc.reshape(B, S, H)
